# revision 18
# baseline (speedup 1.0000x reference)
"""DecoderTreeRNN Trainium2 kernel.

Computes: h0 = relu(encoding); expand a depth-`depth` binary tree with two
zero-input GRU cells (left/right); project every leaf hidden state with W_out
and take log_softmax over the vocab.

Strategy: pure data parallel over 8 NeuronCores (batch sharded), GRU weights
and the output projection replicated.  On-core layout is transposed
([hidden-chunk on partitions, tokens on the free dim]) so all matmuls
contract over partitions and the softmax reduction runs along the free dim.

All matmuls run in fp8 (e4m3, unscaled: |h|<~5, |W|<~0.7 are far inside
e4m3's +-240 range) with DoubleRow perf mode - two 128-deep k-tiles per PE
pass.  The GRU hidden state is stored fp8; gate activations stay bf16/fp32.
y and the output are bf16 (host upcasts to fp32).

Projection: per 128-token chunk, vocab [0:HB) rotates through 2-bank PSUM
tiles (DVE drains with bias, ACT exp+accumulates), while the tail [HB:V)
stays in PSUM with its bias pre-added by a rank-1 ones x b_out matmul, so
after c = ln(sum(exp)) ACT can fuse drain+subtract as Identity(ps + (-c)).
Two phases are sequential and separately scoped so each gets all 8 PSUM
banks.  Output chunks are grouped in NODE order (the last GRU level's
h' = n + z*(h-n) add scatters straight into the fp8 projection operand,
bit-reversal happens in the store's HBM address).
"""

import os
import sys
from contextlib import ExitStack

import numpy as np

for _p in ("/opt/trn_rl_repo", "/root/.axon_site/_ro/trn_rl_repo"):
    if os.path.isdir(_p) and _p not in sys.path:
        sys.path.insert(0, _p)

import ml_dtypes

N_CORES = 8
P = 128
TTILE = 512  # token tile for GRU matmuls (max fp32 moving free dim)
NBF = 512  # fp32 elements per PSUM bank
VGW = 2 * NBF  # vocab group width (2 PSUM banks; 3 rotating slots)
HOLDW = 1024  # vocab tail kept in PSUM until the softmax constant is known
SUB_ACT = int(os.environ.get("TREERNN_SUB_ACT", "2500"))
GPS_DMA = os.environ.get("TREERNN_GPS_DMA", "1") == "1"

# Set by test harness to capture a profile on the next kernel() call.
TRACE = False
SIM_SAFE_DMA = False
LAST_EXEC_NS = None
LAST_RESULTS = None

_COMPILE_CACHE = {}


def _bitrev(x, bits):
    r = 0
    for _ in range(bits):
        r = (r << 1) | (x & 1)
        x >>= 1
    return r


def _numpy_reference(encoding, W_hh_l, b_ih_l, b_hh_l, W_hh_r, b_ih_r, b_hh_r,
                     W_out, b_out, depth):
    def gru(h, W, b_ih, b_hh):
        Hd = h.shape[-1]
        gh = h @ W.T + b_hh
        r = 1.0 / (1.0 + np.exp(-(b_ih[:Hd] + gh[..., :Hd])))
        z = 1.0 / (1.0 + np.exp(-(b_ih[Hd:2 * Hd] + gh[..., Hd:2 * Hd])))
        n = np.tanh(b_ih[2 * Hd:] + r * gh[..., 2 * Hd:])
        return (1.0 - z) * n + z * h

    h = np.maximum(encoding, 0.0)[:, None, :]
    for _ in range(depth):
        left = gru(h, W_hh_l, b_ih_l, b_hh_l)
        right = gru(h, W_hh_r, b_ih_r, b_hh_r)
        h = np.stack([left, right], axis=2).reshape(h.shape[0], -1, h.shape[-1])
    logits = h @ W_out.T + b_out
    m = logits.max(axis=-1, keepdims=True)
    e = np.exp(logits - m)
    return (logits - m) - np.log(e.sum(axis=-1, keepdims=True))


def _patch_act_tables(bacc, mybir):
    """Constrain the ACT table-set chooser so the GRU phase and the
    projection phase each stick to ONE set (2 loads total).  Only the
    chooser's view is filtered; the runtime tables are the real (full)
    sets, so execution is unchanged."""
    from concourse import hw_specs
    AF = mybir.ActivationFunctionType
    orig = hw_specs.get_activation_tables
    if getattr(bacc.get_activation_tables, "_treernn_patch", False):
        return
    keep = {
        "sigmoid_and_others": {AF.Sigmoid, AF.Tanh, AF.Relu},
        "natural_log_exp_and_others": {AF.Exp, AF.Ln, AF.Identity, AF.Copy},
    }
    controlled = set().union(*keep.values())

    def patched(arch):
        tabs = {k: set(v) for k, v in orig(arch).items()}
        for name, s in tabs.items():
            s -= controlled
            s |= keep.get(name, set())
        return tabs

    patched._treernn_patch = True
    bacc.get_activation_tables = patched


def _build(Bc, H, V, depth):
    """Build + compile the single-core SPMD program (identical on all cores)."""
    import concourse.bass as bass  # noqa: F401
    import concourse.tile as tile
    from concourse import bacc, mybir

    f32 = mybir.dt.float32
    bf16 = mybir.dt.bfloat16
    f8 = mybir.dt.float8e4
    AF = mybir.ActivationFunctionType
    OP = mybir.AluOpType
    DR = mybir.MatmulPerfMode.DoubleRow
    _patch_act_tables(bacc, mybir)

    KH = H // P
    H3 = 3 * H
    L = 1 << depth
    TOK = Bc * L
    NTC = (TOK + P - 1) // P
    NLL = max(1, min(P // Bc, L))  # tree nodes (leaves) per token chunk

    HB = V - HOLDW  # start of the PSUM-held vocab tail
    vgroups, pos = [], 0
    while pos < HB:  # PSUM drain granularity (rotating region only)
        w = min(VGW, HB - pos)
        vgroups.append((pos, w))
        pos += w
    e_split = min(4 * VGW, HB)  # exp/accumulate granularity (y region)
    egroups = [(0, e_split)] + ([(e_split, HB - e_split)] if HB > e_split else [])
    EW = max(w for _, w in egroups)
    NEG = len(egroups) + 1  # + the held tail's accumulator

    nc = bacc.Bacc("TRN2", target_bir_lowering=False, debug=False,
                   num_devices=N_CORES)

    enc_d = nc.dram_tensor("enc_t", [P, KH, Bc], f32, kind="ExternalInput").ap()
    whh_d = {s: nc.dram_tensor(f"whht_{s}", [P, KH, H3], f8,
                               kind="ExternalInput").ap() for s in "lr"}
    # packed per-side biases: cols [0:2K]=sigmoid(r,z), [2K:3K]=tanh, [3K:4K]=n_hh
    bias_d = {s: nc.dram_tensor(f"bias_{s}", [P, 4 * KH], f32,
                                kind="ExternalInput").ap() for s in "lr"}
    wout_d = nc.dram_tensor("woutt", [P, KH, V], f8, kind="ExternalInput").ap()
    bout_d = nc.dram_tensor("bout", [P, V], bf16, kind="ExternalInput").ap()
    bout8_d = nc.dram_tensor("bout8", [P, 2, HOLDW], f8,
                             kind="ExternalInput").ap()
    out_d = nc.dram_tensor("out", [Bc, L, V], bf16, kind="ExternalOutput").ap()

    import bass_rust as _br

    with tile.TileContext(nc) as tc, ExitStack() as ctx:
        constp = ctx.enter_context(tc.tile_pool(name="const", bufs=1))
        ht2p = ctx.enter_context(tc.tile_pool(name="ht2", bufs=1))
        ht2 = ht2p.tile([P, KH, TOK], f8)
        wvep = ctx.enter_context(tc.tile_pool(name="wout_early", bufs=1))
        bop = ctx.enter_context(tc.tile_pool(name="bout", bufs=1))

        # --- input staging: enc first (it gates the first relu+matmuls),
        # then biases + GRU weights on the sync queue; the big projection
        # constants go on the (otherwise idle) gpsimd queue.
        enc_sb = constp.tile([P, KH, Bc], f32, name="enc_stage")
        nc.sync.dma_start(out=enc_sb, in_=enc_d)
        bsig, btanh, bnhh = {}, {}, {}
        for s in "lr":
            bt = constp.tile([P, 4 * KH], f32, name=f"bias{s}")
            nc.sync.dma_start(out=bt, in_=bias_d[s])
            bsig[s] = bt[:, :2 * KH]
            btanh[s] = bt[:, 2 * KH:3 * KH]
            bnhh[s] = bt[:, 3 * KH:]
        whh = {}
        for s in "lr":
            w = constp.tile([P, KH, H3], f8, name=f"whh{s}")
            nc.sync.dma_start(out=w, in_=whh_d[s])
            whh[s] = w

        weng = nc.gpsimd if GPS_DMA else nc.sync
        ones8 = bop.tile([P, 2, P], f8, name="ones8")
        nc.gpsimd.memset(ones8, 1.0)
        bout_sb = bop.tile([P, HB], bf16)
        weng.dma_start(out=bout_sb, in_=bout_d[:, :HB])
        bout8_sb = bop.tile([P, 2, HOLDW], f8, name="bout8")
        weng.dma_start(out=bout8_sb, in_=bout8_d)
        wv = []
        for vg, (vs, vw) in enumerate(vgroups):
            wt = wvep.tile([P, KH, vw], f8, name=f"wv{vg}")
            weng.dma_start(out=wt, in_=wout_d[:, :, vs:vs + vw])
            wv.append(wt)
        wvh = wvep.tile([P, KH, HOLDW], f8, name="wvh")
        weng.dma_start(out=wvh, in_=wout_d[:, :, HB:])

        # ---------------- GRU tree expansion (fp8 DoubleRow) --------------
        with tc.tile_pool(name="gh", bufs=1) as ghp, \
             tc.tile_pool(name="gact", bufs=2) as gap, \
             tc.tile_pool(name="gactd", bufs=1) as gdp, \
             tc.tile_pool(name="gpsum", bufs=8, space="PSUM") as gpp:
            h0 = ghp.tile([P, KH, Bc], f8, name="h_l0")
            nc.scalar.activation(out=h0, in_=enc_sb, func=AF.Relu)

            def gru_tile(s, si, t, t0, h_cur, h_nxt):
                """One (side, token-tile) step.  h_nxt=None on the last
                level: the final add scatters straight into ht2 (fp8),
                chunk-grouped in NODE order."""
                tt = min(TTILE, t - t0)
                hs = h_cur[:, :, t0:t0 + tt]
                r_sb = gap.tile([P, KH, TTILE], bf16, name="g_r")[:, :, :tt]
                z_sb = gap.tile([P, KH, TTILE], bf16, name="g_z")[:, :, :tt]
                n_sb = gap.tile([P, KH, TTILE], bf16, name="g_n")[:, :, :tt]
                d_sb = gdp.tile([P, KH, TTILE], bf16, name="g_d")[:, :, :tt]
                for gi in range(3):  # r, z, n
                    for gc in range(KH):
                        ps = gpp.tile([P, TTILE], f32, name="g_ps")[:, :tt]
                        col = gi * H + gc * P
                        for kp in range(KH // 2):
                            nc.tensor.matmul(
                                ps,
                                lhsT=whh[s][:, 2 * kp:2 * kp + 2,
                                            col:col + P],
                                rhs=hs[:, 2 * kp:2 * kp + 2, :],
                                start=(kp == 0), stop=(kp == KH // 2 - 1),
                                perf_mode=DR)
                        if gi == 0:
                            nc.scalar.activation(
                                out=r_sb[:, gc, :], in_=ps,
                                func=AF.Sigmoid, bias=bsig[s][:, gc:gc + 1])
                        elif gi == 1:
                            nc.scalar.activation(
                                out=z_sb[:, gc, :], in_=ps,
                                func=AF.Sigmoid,
                                bias=bsig[s][:, KH + gc:KH + gc + 1])
                        else:
                            # n_pre = (gh_n + b_hh_n) * r
                            nc.vector.scalar_tensor_tensor(
                                out=n_sb[:, gc, :], in0=ps,
                                scalar=bnhh[s][:, gc:gc + 1],
                                in1=r_sb[:, gc, :],
                                op0=OP.add, op1=OP.mult)
                            nc.scalar.activation(
                                out=n_sb[:, gc, :], in_=n_sb[:, gc, :],
                                func=AF.Tanh, bias=btanh[s][:, gc:gc + 1])
                # h' = n + z * (h - n)
                nc.vector.tensor_tensor(d_sb, hs, n_sb, OP.subtract)
                nc.vector.tensor_tensor(d_sb, d_sb, z_sb, OP.mult)
                if h_nxt is not None:
                    soff = si * t
                    nc.vector.tensor_tensor(
                        h_nxt[:, :, soff + t0:soff + t0 + tt],
                        d_sb, n_sb, OP.add)
                    return
                for j in range(tt // Bc):
                    node = (si * t + t0) // Bc + j
                    tcp, ll = node // NLL, node % NLL
                    base = tcp * P + ll
                    nc.vector.tensor_tensor(
                        ht2[:, :, base:base + (Bc - 1) * NLL + 1:NLL],
                        d_sb[:, :, j * Bc:(j + 1) * Bc],
                        n_sb[:, :, j * Bc:(j + 1) * Bc], OP.add)

            h_cur = h0
            for lvl in range(depth):
                t = Bc << lvl
                last = lvl == depth - 1
                h_nxt = None if last else ghp.tile(
                    [P, KH, 2 * t], f8, name=f"h_l{lvl + 1}")
                for si, s in enumerate("lr"):
                    for t0 in range(0, t, TTILE):
                        gru_tile(s, si, t, t0, h_cur, h_nxt)
                h_cur = h_nxt

        # ---------------- projection + log_softmax ------------------------
        with tc.tile_pool(name="ypool", bufs=2) as yp, \
             tc.tile_pool(name="stat", bufs=2) as stp, \
             tc.tile_pool(name="escratch", bufs=1) as esp, \
             tc.tile_pool(name="ppsum", bufs=3, space="PSUM") as ppp, \
             tc.tile_pool(name="phold", bufs=1, space="PSUM") as php:
            dma_engines = [nc.sync, nc.gpsimd] if GPS_DMA else [nc.sync]

            def out_dma(tci, pc, y, v0, v1):
                """Per-leaf stores; chunk tci holds nodes [tci*NLL ..) at
                interleaved partitions p = b*NLL + ll; the HBM leaf row is
                the bit-reversal of the node.  Descriptors alternate
                between the sync and gpsimd queues."""
                nll = pc // Bc
                for ll in range(nll):
                    leaf = _bitrev(tci * nll + ll, depth)
                    eng = dma_engines[ll % len(dma_engines)]
                    eng.dma_start(out=out_d[:, leaf, v0:v1],
                                  in_=y[ll:pc:nll, v0:v1])

            def emit_tail(st, after_tt=None, fine=False):
                """Softmax tail: c = ln(sum); y region gets out = y - c
                (ACT Identity slice + DVE 2-byte fast path), the PSUM-held
                tail drains+subtracts in one ACT Identity(ps + (-c))."""
                tci, pc, y, ph, sums, cs = st
                nc.vector.tensor_reduce(out=cs[:, 0:1], in_=sums,
                                        axis=mybir.AxisListType.X, op=OP.add)
                nc.scalar.activation(out=cs[:, 0:1], in_=cs[:, 0:1],
                                     func=AF.Ln)
                nc.vector.tensor_scalar(out=cs[:, 1:2], in0=cs[:, 0:1],
                                        scalar1=-1.0, scalar2=None,
                                        op0=OP.mult)
                if fine:
                    nq = 6
                    qs = [HB * i // nq for i in range(nq + 1)]
                    for i in range(nq):
                        a, b = qs[i], qs[i + 1]
                        if i % 2 == 0:
                            nc.scalar.activation(out=y[:, a:b], in_=y[:, a:b],
                                                 func=AF.Identity,
                                                 bias=cs[:, 1:2])
                        else:
                            nc.vector.tensor_scalar(out=y[:, a:b],
                                                    in0=y[:, a:b],
                                                    scalar1=cs[:, 0:1],
                                                    scalar2=None,
                                                    op0=OP.subtract)
                        out_dma(tci, pc, y, a, b)
                    nc.scalar.activation(out=y[:, HB:], in_=ph,
                                         func=AF.Identity, bias=cs[:, 1:2])
                    out_dma(tci, pc, y, HB, V)
                    return
                sub = nc.vector.tensor_scalar(out=y[:, SUB_ACT:HB],
                                              in0=y[:, SUB_ACT:HB],
                                              scalar1=cs[:, 0:1],
                                              scalar2=None, op0=OP.subtract)
                if after_tt is not None:
                    # Must NOT precede the next chunk's first drain in the
                    # VectorE FIFO (that stalls the PSUM-slot recycle).
                    _br.add_dep_helper(
                        sub.ins, after_tt.ins, sync=False,
                        reason="tail subtract yields to next drain")
                mid = (SUB_ACT + HB) // 2
                out_dma(tci, pc, y, SUB_ACT, mid)
                out_dma(tci, pc, y, mid, HB)
                nc.scalar.activation(out=y[:, :SUB_ACT], in_=y[:, :SUB_ACT],
                                     func=AF.Identity, bias=cs[:, 1:2])
                out_dma(tci, pc, y, 0, SUB_ACT)
                nc.scalar.activation(out=y[:, HB:], in_=ph,
                                     func=AF.Identity, bias=cs[:, 1:2])
                out_dma(tci, pc, y, HB, V)

            state = {"pending": None}

            def proj_chunk(tci):
                pc = min(P, TOK - tci * P)  # tokens in this chunk
                y = yp.tile([P, V], bf16, name="y")[:pc]
                sums = stp.tile([P, NEG], f32, name="sums")[:pc]
                cs = stp.tile([P, 2], f32, name="cs")[:pc]
                ei = 0
                first_tt = None
                for vg, (vs, vw) in enumerate(vgroups):
                    ps = ppp.tile([P, VGW], f32, name="p_vg")[:pc, :vw]
                    # k-pair outer, vocab tile inner: the stationary operand
                    # (token chunk) is reused across the vocab group
                    for kp in range(KH // 2):
                        for vt0 in range(0, vw, NBF):
                            w = min(NBF, vw - vt0)
                            nc.tensor.matmul(
                                ps[:, vt0:vt0 + w],
                                lhsT=ht2[:, 2 * kp:2 * kp + 2,
                                         tci * P:tci * P + pc],
                                rhs=wv[vg][:, 2 * kp:2 * kp + 2,
                                           vt0:vt0 + w],
                                start=(kp == 0), stop=(kp == KH // 2 - 1),
                                perf_mode=DR)
                    tt = nc.vector.tensor_tensor(
                        y[:, vs:vs + vw], ps, bout_sb[:pc, vs:vs + vw],
                        OP.add)
                    if first_tt is None:
                        first_tt = tt
                    # exp at coarser granularity, reading y (NOT the PSUM
                    # tile - that would stall the slot rotation); only the
                    # accumulated per-partition sum matters
                    es, ew = egroups[ei] if ei < len(egroups) else (None, None)
                    if es is not None and vs + vw >= es + ew:
                        esc = esp.tile([P, EW], bf16, name="e_sc")[:pc, :ew]
                        nc.scalar.activation(out=esc, in_=y[:, es:es + ew],
                                             func=AF.Exp,
                                             accum_out=sums[:, ei:ei + 1])
                        ei += 1
                    # previous chunk's softmax tail, issued after this
                    # chunk's first vocab group
                    if vg == 0 and state["pending"] is not None:
                        emit_tail(state["pending"], after_tt=first_tt)
                # held tail: rank-1 ones x b_out seeds the bias, DoubleRow
                # k-pairs accumulate on top, exp+accumulate runs straight
                # from PSUM; the tile stays live until this chunk's tail.
                ph = php.tile([P, HOLDW], f32, name="p_hold")[:pc]
                for vt0 in range(0, HOLDW, NBF):
                    w = min(NBF, HOLDW - vt0)
                    nc.tensor.matmul(ph[:, vt0:vt0 + w],
                                     lhsT=ones8[:, :, :pc],
                                     rhs=bout8_sb[:, :, vt0:vt0 + w],
                                     start=True, stop=False, perf_mode=DR)
                for kp in range(KH // 2):
                    for vt0 in range(0, HOLDW, NBF):
                        w = min(NBF, HOLDW - vt0)
                        nc.tensor.matmul(
                            ph[:, vt0:vt0 + w],
                            lhsT=ht2[:, 2 * kp:2 * kp + 2,
                                     tci * P:tci * P + pc],
                            rhs=wvh[:, 2 * kp:2 * kp + 2, vt0:vt0 + w],
                            start=False, stop=(kp == KH // 2 - 1),
                            perf_mode=DR)
                esch = esp.tile([P, HOLDW], bf16, name="e_sch")[:pc]
                nc.scalar.activation(out=esch, in_=ph, func=AF.Exp,
                                     accum_out=sums[:, NEG - 1:NEG])
                state["pending"] = (tci, pc, y, ph, sums, cs)

            for tci in range(NTC):
                proj_chunk(tci)
            emit_tail(state["pending"], fine=True)

    nc.compile()
    return nc


def _packed_bias(b_ih, b_hh, H, KH):
    """[P, 4*KH]: sigmoid biases (b_ih+b_hh for r,z), tanh bias (b_ih_n),
    and the pre-multiply n-gate bias (b_hh_n), per 128-row chunk."""
    P = 128
    sig = (b_ih + b_hh)[:2 * H].reshape(2 * KH, P).T
    tanh = b_ih[2 * H:].reshape(KH, P).T
    nhh = b_hh[2 * H:].reshape(KH, P).T
    return np.ascontiguousarray(np.concatenate([sig, tanh, nhh], axis=1))


def _get_compiled(Bc, H, V, depth):
    key = (Bc, H, V, depth)
    if key not in _COMPILE_CACHE:
        _COMPILE_CACHE[key] = _build(Bc, H, V, depth)
    return _COMPILE_CACHE[key]


def kernel(encoding, W_hh_l, b_ih_l, b_hh_l, W_hh_r, b_ih_r, b_hh_r,
           W_out, b_out, depth):
    global LAST_EXEC_NS, LAST_RESULTS
    encoding = np.asarray(encoding, np.float32)
    W_hh = {"l": np.asarray(W_hh_l, np.float32), "r": np.asarray(W_hh_r, np.float32)}
    b_ih = {"l": np.asarray(b_ih_l, np.float32), "r": np.asarray(b_ih_r, np.float32)}
    b_hh = {"l": np.asarray(b_hh_l, np.float32), "r": np.asarray(b_hh_r, np.float32)}
    W_out = np.asarray(W_out, np.float32)
    b_out = np.asarray(b_out, np.float32)
    depth = int(depth)

    B, H = encoding.shape
    V = W_out.shape[0]
    tok = (B // N_CORES) * (1 << depth) if B % N_CORES == 0 else 0
    if (depth < 1 or B % N_CORES or H % P or P % (B // N_CORES)
            or (tok % P != 0 and tok > P) or V <= 2 * HOLDW):
        return _numpy_reference(encoding, W_hh["l"], b_ih["l"], b_hh["l"],
                                W_hh["r"], b_ih["r"], b_hh["r"],
                                W_out, b_out, depth).astype(np.float32)

    Bc = B // N_CORES
    KH = H // P
    bf16 = ml_dtypes.bfloat16
    f8 = ml_dtypes.float8_e4m3

    nc = _get_compiled(Bc, H, V, depth)

    # device layouts are [P(partition), KH, x]: H index = k*P + p -> axes (p, k)
    woutt = np.ascontiguousarray(
        W_out.T.astype(f8).reshape(KH, P, V).transpose(1, 0, 2))
    bout_b = np.ascontiguousarray(
        np.broadcast_to(b_out.astype(bf16)[None, :], (P, V)))
    bout8 = np.zeros((P, 2, HOLDW), f8)
    bout8[0, 0, :] = b_out[V - HOLDW:].astype(f8)
    shared = {"woutt": woutt, "bout": bout_b, "bout8": bout8}
    for s in "lr":
        shared[f"whht_{s}"] = np.ascontiguousarray(
            W_hh[s].T.astype(f8).reshape(KH, P, 3 * H).transpose(1, 0, 2))
        shared[f"bias_{s}"] = _packed_bias(b_ih[s], b_hh[s], H, KH)

    encT = encoding.T  # [H, B]
    in_maps = []
    for c in range(N_CORES):
        enc_c = np.ascontiguousarray(
            encT[:, c * Bc:(c + 1) * Bc].reshape(KH, P, Bc).transpose(1, 0, 2))
        in_maps.append({"enc_t": enc_c, **shared})

    from concourse import bass_utils
    kw = {}
    if TRACE:
        kw["tmpdir"] = os.environ.get("BASS_TRACE_DIR") or None
    res = bass_utils.run_bass_kernel_spmd(
        nc, in_maps, core_ids=list(range(N_CORES)), trace=TRACE, **kw)
    LAST_EXEC_NS = res.exec_time_ns
    LAST_RESULTS = res
    out = np.concatenate([np.asarray(r["out"]) for r in res.results], axis=0)
    return np.ascontiguousarray(out.astype(np.float32))


# revision 19
# speedup vs baseline: 1.2920x; 1.2920x over previous
"""DecoderTreeRNN Trainium2 kernel.

Computes: h0 = relu(encoding); expand a depth-`depth` binary tree with two
zero-input GRU cells (left/right); project every leaf hidden state with W_out
and take log_softmax over the vocab.

Strategy: pure data parallel over 8 NeuronCores (batch sharded), GRU weights
and the output projection replicated.  On-core layout is transposed
([hidden-chunk on partitions, tokens on the free dim]) so all matmuls
contract over partitions and the softmax reduction runs along the free dim.

The GRU runs in bf16 (fp8 recurrence measured slower: it breaks the DVE
2-byte fast paths and deepens PE power-throttling).  The projection runs in
fp8 e4m3 (unscaled - |h|<~5, |W_out|<~0.7 sit far inside +-240) with
DoubleRow perf mode: two 128-deep k-tiles per PE pass.  y and the output
are bf16; the host upcasts to fp32.

The last GRU level's h' = n + z*(h-n) add writes straight into the fp8
projection operand (cast fused with the per-node scatter); output chunks
are grouped in NODE order and the store bit-reverses when picking the HBM
leaf row.  Output DMA descriptors alternate between the sync and gpsimd
queues so neither serializes the store stream.
"""

import os
import sys
from contextlib import ExitStack

import numpy as np

for _p in ("/opt/trn_rl_repo", "/root/.axon_site/_ro/trn_rl_repo"):
    if os.path.isdir(_p) and _p not in sys.path:
        sys.path.insert(0, _p)

import ml_dtypes

N_CORES = 8
P = 128
TTILE = 512  # token tile for GRU matmuls (max fp32 moving free dim)
NBF = 512  # fp32 elements per PSUM bank
VGW = 4 * NBF  # vocab group width (4 PSUM banks; 2 rotating slots)
EGW = 5000  # exp granularity (decoupled from the PSUM drain)
SUB_ACT = int(os.environ.get("TREERNN_SUB_ACT", "4352"))
GPS_DMA = os.environ.get("TREERNN_GPS_DMA", "1") == "1"

# Set by test harness to capture a profile on the next kernel() call.
TRACE = False
SIM_SAFE_DMA = False
LAST_EXEC_NS = None
LAST_RESULTS = None

_COMPILE_CACHE = {}


def _bitrev(x, bits):
    r = 0
    for _ in range(bits):
        r = (r << 1) | (x & 1)
        x >>= 1
    return r


def _numpy_reference(encoding, W_hh_l, b_ih_l, b_hh_l, W_hh_r, b_ih_r, b_hh_r,
                     W_out, b_out, depth):
    def gru(h, W, b_ih, b_hh):
        Hd = h.shape[-1]
        gh = h @ W.T + b_hh
        r = 1.0 / (1.0 + np.exp(-(b_ih[:Hd] + gh[..., :Hd])))
        z = 1.0 / (1.0 + np.exp(-(b_ih[Hd:2 * Hd] + gh[..., Hd:2 * Hd])))
        n = np.tanh(b_ih[2 * Hd:] + r * gh[..., 2 * Hd:])
        return (1.0 - z) * n + z * h

    h = np.maximum(encoding, 0.0)[:, None, :]
    for _ in range(depth):
        left = gru(h, W_hh_l, b_ih_l, b_hh_l)
        right = gru(h, W_hh_r, b_ih_r, b_hh_r)
        h = np.stack([left, right], axis=2).reshape(h.shape[0], -1, h.shape[-1])
    logits = h @ W_out.T + b_out
    m = logits.max(axis=-1, keepdims=True)
    e = np.exp(logits - m)
    return (logits - m) - np.log(e.sum(axis=-1, keepdims=True))


def _patch_act_tables(bacc, mybir):
    """Constrain the ACT table-set chooser so the GRU phase and the
    projection phase each stick to ONE set (2 loads total).  Only the
    chooser's view is filtered; the runtime tables are the real (full)
    sets, so execution is unchanged."""
    from concourse import hw_specs
    AF = mybir.ActivationFunctionType
    orig = hw_specs.get_activation_tables
    if getattr(bacc.get_activation_tables, "_treernn_patch", False):
        return
    keep = {
        "sigmoid_and_others": {AF.Sigmoid, AF.Tanh, AF.Relu},
        "natural_log_exp_and_others": {AF.Exp, AF.Ln, AF.Identity, AF.Copy},
    }
    controlled = set().union(*keep.values())

    def patched(arch):
        tabs = {k: set(v) for k, v in orig(arch).items()}
        for name, s in tabs.items():
            s -= controlled
            s |= keep.get(name, set())
        return tabs

    patched._treernn_patch = True
    bacc.get_activation_tables = patched


def _build(Bc, H, V, depth):
    """Build + compile the single-core SPMD program (identical on all cores)."""
    import concourse.bass as bass  # noqa: F401
    import concourse.tile as tile
    from concourse import bacc, mybir

    f32 = mybir.dt.float32
    bf16 = mybir.dt.bfloat16
    f8 = mybir.dt.float8e4
    AF = mybir.ActivationFunctionType
    OP = mybir.AluOpType
    DR = mybir.MatmulPerfMode.DoubleRow
    _patch_act_tables(bacc, mybir)

    KH = H // P
    H3 = 3 * H
    L = 1 << depth
    TOK = Bc * L
    NTC = (TOK + P - 1) // P
    NLL = max(1, min(P // Bc, L))  # tree nodes (leaves) per token chunk

    def _chunks(width):
        out, pos = [], 0
        while pos < V:
            w = min(width, V - pos)
            out.append((pos, w))
            pos += w
        return out

    vgroups = _chunks(VGW)   # PSUM drain granularity
    egroups = _chunks(EGW)   # exp/accumulate granularity
    NEG = len(egroups)

    nc = bacc.Bacc("TRN2", target_bir_lowering=False, debug=False,
                   num_devices=N_CORES)

    enc_d = nc.dram_tensor("enc_t", [P, KH, Bc], f32, kind="ExternalInput").ap()
    whh_d = {s: nc.dram_tensor(f"whht_{s}", [P, KH, H3], bf16,
                               kind="ExternalInput").ap() for s in "lr"}
    # packed per-side biases: cols [0:2K]=sigmoid(r,z), [2K:3K]=tanh, [3K:4K]=n_hh
    bias_d = {s: nc.dram_tensor(f"bias_{s}", [P, 4 * KH], f32,
                                kind="ExternalInput").ap() for s in "lr"}
    wout_d = nc.dram_tensor("woutt", [P, KH, V], f8, kind="ExternalInput").ap()
    bout_d = nc.dram_tensor("bout", [P, V], bf16, kind="ExternalInput").ap()
    out_d = nc.dram_tensor("out", [Bc, L, V], bf16, kind="ExternalOutput").ap()

    import bass_rust as _br

    with tile.TileContext(nc) as tc, ExitStack() as ctx:
        constp = ctx.enter_context(tc.tile_pool(name="const", bufs=1))
        ht2p = ctx.enter_context(tc.tile_pool(name="ht2", bufs=1))
        ht2 = ht2p.tile([P, KH, TOK], f8)
        wvep = ctx.enter_context(tc.tile_pool(name="wout_early", bufs=1))
        bop = ctx.enter_context(tc.tile_pool(name="bout", bufs=1))

        # --- input staging: enc first (it gates the first relu+matmuls),
        # then biases + GRU weights on the sync queue; the big projection
        # constants go on the (otherwise idle) gpsimd queue.
        enc_sb = constp.tile([P, KH, Bc], f32, name="enc_stage")
        nc.sync.dma_start(out=enc_sb, in_=enc_d)
        bsig, btanh, bnhh = {}, {}, {}
        for s in "lr":
            bt = constp.tile([P, 4 * KH], f32, name=f"bias{s}")
            nc.sync.dma_start(out=bt, in_=bias_d[s])
            bsig[s] = bt[:, :2 * KH]
            btanh[s] = bt[:, 2 * KH:3 * KH]
            bnhh[s] = bt[:, 3 * KH:]
        whh = {}
        for s in "lr":
            w = constp.tile([P, KH, H3], bf16, name=f"whh{s}")
            nc.sync.dma_start(out=w, in_=whh_d[s])
            whh[s] = w

        weng = nc.gpsimd if GPS_DMA else nc.sync
        bout_sb = bop.tile([P, V], bf16)
        weng.dma_start(out=bout_sb, in_=bout_d)
        wv = []
        for vg, (vs, vw) in enumerate(vgroups):
            wt = wvep.tile([P, KH, vw], f8, name=f"wv{vg}")
            weng.dma_start(out=wt, in_=wout_d[:, :, vs:vs + vw])
            wv.append(wt)

        # ---------------- GRU tree expansion (bf16) -----------------------
        with tc.tile_pool(name="gh", bufs=1) as ghp, \
             tc.tile_pool(name="gact", bufs=2) as gap, \
             tc.tile_pool(name="gactd", bufs=1) as gdp, \
             tc.tile_pool(name="gpsum", bufs=8, space="PSUM") as gpp:
            h0 = ghp.tile([P, KH, Bc], bf16, name="h_l0")
            nc.scalar.activation(out=h0, in_=enc_sb, func=AF.Relu)

            def gru_tile(s, si, t, t0, h_cur, h_nxt):
                """One (side, token-tile) step.  h_nxt=None on the last
                level: the final add scatters straight into ht2 (fp8),
                chunk-grouped in NODE order."""
                tt = min(TTILE, t - t0)
                hs = h_cur[:, :, t0:t0 + tt]
                r_sb = gap.tile([P, KH, TTILE], bf16, name="g_r")[:, :, :tt]
                z_sb = gap.tile([P, KH, TTILE], bf16, name="g_z")[:, :, :tt]
                n_sb = gap.tile([P, KH, TTILE], bf16, name="g_n")[:, :, :tt]
                d_sb = gdp.tile([P, KH, TTILE], bf16, name="g_d")[:, :, :tt]
                for gi in range(3):  # r, z, n
                    for gc in range(KH):
                        ps = gpp.tile([P, TTILE], f32, name="g_ps")[:, :tt]
                        col = gi * H + gc * P
                        for k in range(KH):
                            nc.tensor.matmul(
                                ps,
                                lhsT=whh[s][:, k, col:col + P],
                                rhs=hs[:, k, :],
                                start=(k == 0), stop=(k == KH - 1))
                        if gi == 0:
                            nc.scalar.activation(
                                out=r_sb[:, gc, :], in_=ps,
                                func=AF.Sigmoid, bias=bsig[s][:, gc:gc + 1])
                        elif gi == 1:
                            nc.scalar.activation(
                                out=z_sb[:, gc, :], in_=ps,
                                func=AF.Sigmoid,
                                bias=bsig[s][:, KH + gc:KH + gc + 1])
                        else:
                            # n_pre = (gh_n + b_hh_n) * r
                            nc.vector.scalar_tensor_tensor(
                                out=n_sb[:, gc, :], in0=ps,
                                scalar=bnhh[s][:, gc:gc + 1],
                                in1=r_sb[:, gc, :],
                                op0=OP.add, op1=OP.mult)
                            nc.scalar.activation(
                                out=n_sb[:, gc, :], in_=n_sb[:, gc, :],
                                func=AF.Tanh, bias=btanh[s][:, gc:gc + 1])
                # h' = n + z * (h - n)
                nc.vector.tensor_tensor(d_sb, hs, n_sb, OP.subtract)
                nc.vector.tensor_tensor(d_sb, d_sb, z_sb, OP.mult)
                if h_nxt is not None:
                    soff = si * t
                    nc.vector.tensor_tensor(
                        h_nxt[:, :, soff + t0:soff + t0 + tt],
                        d_sb, n_sb, OP.add)
                    return
                for j in range(tt // Bc):
                    node = (si * t + t0) // Bc + j
                    tcp, ll = node // NLL, node % NLL
                    base = tcp * P + ll
                    nc.vector.tensor_tensor(
                        ht2[:, :, base:base + (Bc - 1) * NLL + 1:NLL],
                        d_sb[:, :, j * Bc:(j + 1) * Bc],
                        n_sb[:, :, j * Bc:(j + 1) * Bc], OP.add)

            h_cur = h0
            for lvl in range(depth):
                t = Bc << lvl
                last = lvl == depth - 1
                h_nxt = None if last else ghp.tile(
                    [P, KH, 2 * t], bf16, name=f"h_l{lvl + 1}")
                for si, s in enumerate("lr"):
                    for t0 in range(0, t, TTILE):
                        gru_tile(s, si, t, t0, h_cur, h_nxt)
                h_cur = h_nxt

        # ---------------- projection + log_softmax ------------------------
        with tc.tile_pool(name="ypool", bufs=3) as yp, \
             tc.tile_pool(name="stat", bufs=3) as stp, \
             tc.tile_pool(name="escratch", bufs=1) as esp, \
             tc.tile_pool(name="ppsum", bufs=2, space="PSUM") as ppp:
            dma_engines = [nc.sync, nc.gpsimd] if GPS_DMA else [nc.sync]

            def out_dma(tci, pc, y, v0, v1):
                """Per-leaf stores; chunk tci holds nodes [tci*NLL ..) at
                interleaved partitions p = b*NLL + ll (strides across all
                16 SBUF port groups); the HBM leaf row is the bit-reversal
                of the node."""
                nll = pc // Bc
                for ll in range(nll):
                    leaf = _bitrev(tci * nll + ll, depth)
                    eng = dma_engines[ll % len(dma_engines)]
                    eng.dma_start(out=out_d[:, leaf, v0:v1],
                                  in_=y[ll:pc:nll, v0:v1])

            def emit_tail(st, after_tt=None, fine=False):
                """Softmax tail for a finished chunk: c = ln(sum), out -= c.
                fine=True (last chunk): alternate small ACT/DVE slices with
                immediate stores so the kernel-end drain isn't gated on one
                big subtract + one big DMA."""
                tci, pc, y, sums, cs = st
                nc.vector.tensor_reduce(out=cs[:, 0:1], in_=sums,
                                        axis=mybir.AxisListType.X, op=OP.add)
                nc.scalar.activation(out=cs[:, 0:1], in_=cs[:, 0:1],
                                     func=AF.Ln)
                nc.vector.tensor_scalar(out=cs[:, 1:2], in0=cs[:, 0:1],
                                        scalar1=-1.0, scalar2=None,
                                        op0=OP.mult)
                if fine:
                    nq = 8
                    qs = [V * i // nq for i in range(nq + 1)]
                    for i in range(nq):
                        a, b = qs[i], qs[i + 1]
                        if i % 2 == 0:
                            nc.scalar.activation(out=y[:, a:b], in_=y[:, a:b],
                                                 func=AF.Identity,
                                                 bias=cs[:, 1:2])
                        else:
                            nc.vector.tensor_scalar(out=y[:, a:b],
                                                    in0=y[:, a:b],
                                                    scalar1=cs[:, 0:1],
                                                    scalar2=None,
                                                    op0=OP.subtract)
                        out_dma(tci, pc, y, a, b)
                    return
                if SUB_ACT > 0:
                    nc.scalar.activation(out=y[:, :SUB_ACT],
                                         in_=y[:, :SUB_ACT],
                                         func=AF.Identity, bias=cs[:, 1:2])
                    out_dma(tci, pc, y, 0, SUB_ACT)
                if SUB_ACT < V:
                    # Must NOT precede the next chunk's first drain in the
                    # VectorE FIFO (that stalls the PSUM-slot recycle), so
                    # order it explicitly after that drain.
                    sub = nc.vector.tensor_scalar(out=y[:, SUB_ACT:],
                                                  in0=y[:, SUB_ACT:],
                                                  scalar1=cs[:, 0:1],
                                                  scalar2=None,
                                                  op0=OP.subtract)
                    if after_tt is not None:
                        _br.add_dep_helper(
                            sub.ins, after_tt.ins, sync=False,
                            reason="tail subtract yields to next drain")
                    mid = (SUB_ACT + V) // 2
                    out_dma(tci, pc, y, SUB_ACT, mid)
                    out_dma(tci, pc, y, mid, V)

            pending = None  # previous chunk's tail, pipelined one chunk late
            for tci in range(NTC):
                pc = min(P, TOK - tci * P)  # tokens in this chunk
                y = yp.tile([P, V], bf16, name="y")[:pc]
                sums = stp.tile([P, NEG], f32, name="sums")[:pc]
                cs = stp.tile([P, 2], f32, name="cs")[:pc]
                ei = 0
                first_tt = None
                for vg, (vs, vw) in enumerate(vgroups):
                    ps = ppp.tile([P, VGW], f32, name="p_vg")[:pc, :vw]
                    # k-pair outer, vocab tile inner: the stationary operand
                    # (token chunk) is reused across the whole vocab group
                    for kp in range(KH // 2):
                        for vt0 in range(0, vw, NBF):
                            w = min(NBF, vw - vt0)
                            nc.tensor.matmul(
                                ps[:, vt0:vt0 + w],
                                lhsT=ht2[:, 2 * kp:2 * kp + 2,
                                         tci * P:tci * P + pc],
                                rhs=wv[vg][:, 2 * kp:2 * kp + 2,
                                           vt0:vt0 + w],
                                start=(kp == 0), stop=(kp == KH // 2 - 1),
                                perf_mode=DR)
                    tt = nc.vector.tensor_tensor(
                        y[:, vs:vs + vw], ps, bout_sb[:pc, vs:vs + vw],
                        OP.add)
                    if first_tt is None:
                        first_tt = tt
                    # exp at coarser granularity, reading y (NOT the PSUM
                    # tile - that would stall the slot rotation); only the
                    # accumulated per-partition sum matters
                    es, ew = egroups[ei] if ei < NEG else (None, None)
                    if es is not None and vs + vw >= es + ew:
                        esc = esp.tile([P, EGW], bf16, name="e_sc")[:pc, :ew]
                        nc.scalar.activation(out=esc, in_=y[:, es:es + ew],
                                             func=AF.Exp,
                                             accum_out=sums[:, ei:ei + 1])
                        ei += 1
                    # previous chunk's softmax tail, issued after this
                    # chunk's first vocab group
                    if vg == 0 and pending is not None:
                        emit_tail(pending, after_tt=first_tt)
                pending = (tci, pc, y, sums, cs)
            emit_tail(pending, fine=True)

    nc.compile()
    return nc


def _packed_bias(b_ih, b_hh, H, KH):
    """[P, 4*KH]: sigmoid biases (b_ih+b_hh for r,z), tanh bias (b_ih_n),
    and the pre-multiply n-gate bias (b_hh_n), per 128-row chunk."""
    P = 128
    sig = (b_ih + b_hh)[:2 * H].reshape(2 * KH, P).T
    tanh = b_ih[2 * H:].reshape(KH, P).T
    nhh = b_hh[2 * H:].reshape(KH, P).T
    return np.ascontiguousarray(np.concatenate([sig, tanh, nhh], axis=1))


def _get_compiled(Bc, H, V, depth):
    key = (Bc, H, V, depth)
    if key not in _COMPILE_CACHE:
        _COMPILE_CACHE[key] = _build(Bc, H, V, depth)
    return _COMPILE_CACHE[key]


def kernel(encoding, W_hh_l, b_ih_l, b_hh_l, W_hh_r, b_ih_r, b_hh_r,
           W_out, b_out, depth):
    global LAST_EXEC_NS, LAST_RESULTS
    encoding = np.asarray(encoding, np.float32)
    W_hh = {"l": np.asarray(W_hh_l, np.float32), "r": np.asarray(W_hh_r, np.float32)}
    b_ih = {"l": np.asarray(b_ih_l, np.float32), "r": np.asarray(b_ih_r, np.float32)}
    b_hh = {"l": np.asarray(b_hh_l, np.float32), "r": np.asarray(b_hh_r, np.float32)}
    W_out = np.asarray(W_out, np.float32)
    b_out = np.asarray(b_out, np.float32)
    depth = int(depth)

    B, H = encoding.shape
    V = W_out.shape[0]
    tok = (B // N_CORES) * (1 << depth) if B % N_CORES == 0 else 0
    if (depth < 1 or B % N_CORES or H % P or P % (B // N_CORES)
            or (tok % P != 0 and tok > P)):
        return _numpy_reference(encoding, W_hh["l"], b_ih["l"], b_hh["l"],
                                W_hh["r"], b_ih["r"], b_hh["r"],
                                W_out, b_out, depth).astype(np.float32)

    Bc = B // N_CORES
    KH = H // P
    bf16 = ml_dtypes.bfloat16
    f8 = ml_dtypes.float8_e4m3

    nc = _get_compiled(Bc, H, V, depth)

    # device layouts are [P(partition), KH, x]: H index = k*P + p -> axes (p, k)
    woutt = np.ascontiguousarray(
        W_out.T.astype(f8).reshape(KH, P, V).transpose(1, 0, 2))
    bout_b = np.ascontiguousarray(
        np.broadcast_to(b_out.astype(bf16)[None, :], (P, V)))
    shared = {"woutt": woutt, "bout": bout_b}
    for s in "lr":
        shared[f"whht_{s}"] = np.ascontiguousarray(
            W_hh[s].T.astype(bf16).reshape(KH, P, 3 * H).transpose(1, 0, 2))
        shared[f"bias_{s}"] = _packed_bias(b_ih[s], b_hh[s], H, KH)

    encT = encoding.T  # [H, B]
    in_maps = []
    for c in range(N_CORES):
        enc_c = np.ascontiguousarray(
            encT[:, c * Bc:(c + 1) * Bc].reshape(KH, P, Bc).transpose(1, 0, 2))
        in_maps.append({"enc_t": enc_c, **shared})

    from concourse import bass_utils
    kw = {}
    if TRACE:
        kw["tmpdir"] = os.environ.get("BASS_TRACE_DIR") or None
    res = bass_utils.run_bass_kernel_spmd(
        nc, in_maps, core_ids=list(range(N_CORES)), trace=TRACE, **kw)
    LAST_EXEC_NS = res.exec_time_ns
    LAST_RESULTS = res
    out = np.concatenate([np.asarray(r["out"]) for r in res.results], axis=0)
    return np.ascontiguousarray(out.astype(np.float32))
